# revision 1
# baseline (speedup 1.0000x reference)
"""Trainium2 Bass kernel for nn_Linear_18494129177115 (moe_routing).

Math (reference, fp32):
  base   = x @ W^T                                  [B,T,O]
  logits = x @ Wr^T + lang_bias                     [B,T,E]
  gates  = scatter(softmax(top2(logits)))           [B,T,E]
  h      = x @ A_e^T  (all experts)                 [B,T,E,R]
  out    = base + SCALING * sum_e gates_e * h_e @ B_e^T

Key design points:
- Correctness gate is rel_err < 2e-2, so the heavy GEMMs run as a
  SINGLE bf16 pass (hi parts only) with fp32 PSUM accumulation:
  measured absmax scale-relative error ~3.3e-3 including bf16 output
  staging. This is 3x less PE work than a hi/lo 3-pass split.
- The router is the one place bf16 error could blow up: a top-2
  selection flip on a near-tie token changes gate weights discretely.
  Logits are computed to ~fp32 accuracy with three bf16 products
  (xh@wrh + xh@wrl + xl@wrh, dropping only the ~2^-18 xl@wrl term),
  packed as 2 matmuls per contraction chunk: xh @ [wrh|wrl] (40-wide
  stationary, wrl at col 32 so both PSUM groups sit at legal base
  partitions) and xl @ wrh. Host-sim shows 0/8192 selection flips vs
  the fp32 reference; 1- and 2-term routers flip 7-11 tokens.
- With A_cat = concat_e(A_e) [E*R, D] and B_cat[e*R+r, o] = B[e, o, r],
  the gated LoRA collapses to
    out = x @ W^T + (gates_expanded * (x @ A_cat^T)) @ (SCALING * B_cat),
  two thin matmuls fused into the base GEMM's PSUM accumulation.
- x hi/lo live in 8 per-4-chunk SBUF tiles so the router starts after
  the first 1MB DMA lands instead of waiting for the full 16MB load.
- The language bias (per-core constant row) is folded into the logits
  combine as a per-partition scalar add on the Activation engine.
- All 8 logit transposes land in one PSUM bank; the per-token-tile
  top-2/softmax DVE chains then overlap the h matmuls on the PE.
- Output staged per 512-col tile in bf16 (absmax contribution ~2e-3)
  and written as one 1MB DMA per output tile on the SWDGE ring.

Sharding: data-parallel over tokens, 1024 tokens/core on 8 cores; all
weights replicated; no collectives. Each core's tokens lie in a single
batch row, so the language bias is a per-core constant row (tiny input).
"""

import numpy as np

LANG_BIAS = 5.0
SCALING = 32.0 / 16.0
B_SZ, T_SZ, D_SZ, O_SZ, E_SZ, R_SZ = 4, 2048, 4096, 4096, 8, 16
NCORES = 8
TPC = (B_SZ * T_SZ) // NCORES      # 1024 tokens per core
NT = TPC // 128                    # 8 token tiles per core
NK = D_SZ // 128                   # 32 contraction chunks
NKB = NK // 4                      # 8 x/W-chunk groups (4 kc each)
NO = O_SZ // 512                   # 8 output tiles of 512
ER = E_SZ * R_SZ                   # 128 (expert, rank) pairs
NB = TPC // 512                    # 2 token halves of 512

_CACHE: dict = {}
LAST_RESULT = None


def _build_bass(loop_n: int | None = None):
    import concourse.bacc as bacc
    import concourse.mybir as mybir
    from concourse import tile
    from concourse.masks import make_identity

    f32 = mybir.dt.float32
    bf16 = mybir.dt.bfloat16
    AX = mybir.AxisListType.X
    OP = mybir.AluOpType
    ACT = mybir.ActivationFunctionType

    nc = bacc.Bacc(None, target_bir_lowering=False, debug=False)

    # [2(hi/lo), kc, p, t]
    xt_d = nc.dram_tensor("xt", [2, NK, 128, TPC], bf16, kind="ExternalInput")
    # W^T stream: per (ot, kcb) a [128, 4kc, 512] block, rows contiguous
    wt_d = nc.dram_tensor("wt", [NO, NKB, 128, 4, 512], bf16, kind="ExternalInput")
    # A_cat^T: per kc a [128, ER] block (hi only)
    acat_d = nc.dram_tensor("acat", [NK, 128, ER], bf16, kind="ExternalInput")
    # router weights: wrh in cols 0-7, wrl in cols 32-39
    wrcat_d = nc.dram_tensor("wrcat", [NK, 128, 40], bf16, kind="ExternalInput")
    bcat_d = nc.dram_tensor("bcat", [ER, O_SZ], bf16, kind="ExternalInput")
    bias_d = nc.dram_tensor("biasx", [E_SZ, 1], f32, kind="ExternalInput")
    sel_d = nc.dram_tensor("sel", [E_SZ, ER], bf16, kind="ExternalInput")
    out_d = nc.dram_tensor("out", [NO, 128, NT, 512], bf16, kind="ExternalOutput")

    with tile.TileContext(nc) as tc:
        with (
            tc.tile_pool(name="const", bufs=1) as cpool,
            tc.tile_pool(name="wstream", bufs=3) as wpool,
            tc.tile_pool(name="ostage", bufs=2) as opool,
            tc.tile_pool(name="gate", bufs=2) as gpool,
            tc.tile_pool(name="psum", bufs=8, space="PSUM") as psum,
        ):

            def body(_iv=None):
                # ---- resident inputs; x in per-4kc tiles for fine deps ----
                xh_g = [
                    cpool.tile([128, 4, TPC], bf16, name=f"xh_g{g}")
                    for g in range(NKB)
                ]
                xl_g = [
                    cpool.tile([128, 4, TPC], bf16, name=f"xl_g{g}")
                    for g in range(NKB)
                ]

                def xh(kc):
                    return xh_g[kc // 4][:, kc % 4, :]

                def xl(kc):
                    return xl_g[kc // 4][:, kc % 4, :]

                wrcat_sb = cpool.tile([128, NK, 40], bf16, name="wrcat_sb")
                acat_sb = cpool.tile([128, NK, ER], bf16, name="acat_sb")
                bch_sb = cpool.tile([ER, O_SZ], bf16, name="bch_sb")
                bias_sb = cpool.tile([E_SZ, 1], f32, name="bias_sb")
                sel_sb = cpool.tile([E_SZ, ER], bf16, name="sel_sb")
                ident_sb = cpool.tile([128, 128], f32, name="ident_sb")
                ident8_sb = cpool.tile([8, 8], f32, name="ident8_sb")
                hT_sb = cpool.tile([128, TPC], f32, name="hT_sb")
                ghT_sb = cpool.tile([128, NT, 128], bf16, name="ghT_sb")
                lgT_sb = cpool.tile([E_SZ, TPC], f32, name="lgT_sb")

                # small tensors first so the router isn't queued behind x;
                # bcat (needed only at the end of ot=0) goes last
                nc.scalar.dma_start(
                    wrcat_sb[:], wrcat_d[:].rearrange("k p e -> p k e")
                )
                nc.scalar.dma_start(bias_sb[:], bias_d[:])
                nc.scalar.dma_start(sel_sb[:], sel_d[:])
                nc.scalar.dma_start(
                    acat_sb[:], acat_d[:].rearrange("k p e -> p k e")
                )
                # x hi/lo: 1MB per DMA, alternating HWDGE rings, hi first
                for g in range(NKB):
                    ksl = slice(g * 4, (g + 1) * 4)
                    eng = nc.sync if g % 2 == 0 else nc.scalar
                    eng.dma_start(
                        xh_g[g][:], xt_d[0, ksl].rearrange("k p t -> p k t")
                    )
                    eng = nc.scalar if g % 2 == 0 else nc.sync
                    eng.dma_start(
                        xl_g[g][:], xt_d[1, ksl].rearrange("k p t -> p k t")
                    )
                nc.sync.dma_start(bch_sb[:], bcat_d[:])
                make_identity(nc, ident_sb[:])
                make_identity(nc, ident8_sb[:])

                # ---- phase 1a: router logits^T = [wrh|wrl]^T@xh + wrh^T@xl
                pl16 = [
                    psum.tile([40, 512], f32, tag="bank", name=f"pl16_{t}")
                    for t in range(NB)
                ]
                pl2 = [
                    psum.tile([E_SZ, 512], f32, tag="bank", name=f"pl2_{t}")
                    for t in range(NB)
                ]
                for kc in range(NK):
                    for tb in range(NB):
                        sl = slice(tb * 512, (tb + 1) * 512)
                        nc.tensor.matmul(
                            pl16[tb][:],
                            wrcat_sb[:, kc, :],
                            xh(kc)[:, sl],
                            start=(kc == 0),
                            stop=(kc == NK - 1),
                        )
                        nc.tensor.matmul(
                            pl2[tb][:],
                            wrcat_sb[:, kc, 0:E_SZ],
                            xl(kc)[:, sl],
                            start=(kc == 0),
                            stop=(kc == NK - 1),
                        )
                # combine hi/lo parts + language bias (per-partition scalar)
                for tb in range(NB):
                    sl = slice(tb * 512, (tb + 1) * 512)
                    tsum = gpool.tile([E_SZ, 512], f32, name="tsum")
                    tsum2 = gpool.tile([E_SZ, 512], f32, name="tsum2")
                    nc.scalar.add(tsum[:], pl16[tb][0:E_SZ, :], bias_sb[:])
                    nc.vector.tensor_tensor(
                        tsum2[:], tsum[:], pl16[tb][32:40, :], op=OP.add
                    )
                    nc.vector.tensor_tensor(
                        lgT_sb[:, sl], tsum2[:], pl2[tb][:], op=OP.add
                    )

                # ---- phase 1c-a: all logit transposes into one PSUM bank
                plg_all = psum.tile(
                    [128, NT, E_SZ], f32, tag="bank", name="plg_all"
                )
                for tt in range(NT):
                    ts = slice(tt * 128, (tt + 1) * 128)
                    nc.tensor.transpose(
                        plg_all[:, tt, :], lgT_sb[:, ts], ident8_sb[:]
                    )

                # ---- phase 1b: h^T = A_cat @ x^T (single bf16 pass); the
                # per-tile gating DVE chains below overlap these matmuls
                ph = [
                    psum.tile([128, 512], f32, tag="bank", name=f"ph{t}")
                    for t in range(NB)
                ]
                for kc in range(NK):
                    for tb in range(NB):
                        sl = slice(tb * 512, (tb + 1) * 512)
                        nc.tensor.matmul(
                            ph[tb][:],
                            acat_sb[:, kc, :],
                            xh(kc)[:, sl],
                            start=(kc == 0),
                            stop=(kc == NK - 1),
                        )

                # ---- phase 1c-b: top-2 softmax gates (DVE, overlaps h)
                gates_t = []
                for tt in range(NT):
                    logit = plg_all[:, tt, :]
                    m1 = gpool.tile([128, 1], f32, name="m1")
                    nc.vector.reduce_max(m1[:], logit, axis=AX)
                    mask1 = gpool.tile([128, E_SZ], f32, name="mask1")
                    nc.vector.tensor_scalar(
                        mask1[:], logit, m1[:], None, op0=OP.is_equal
                    )
                    l2 = gpool.tile([128, E_SZ], f32, name="l2")
                    nc.vector.tensor_scalar(
                        l2[:], mask1[:], -1e30, None, op0=OP.mult
                    )
                    nc.vector.tensor_tensor(l2[:], l2[:], logit, op=OP.add)
                    m2 = gpool.tile([128, 1], f32, name="m2")
                    nc.vector.reduce_max(m2[:], l2[:], axis=AX)
                    mask2 = gpool.tile([128, E_SZ], f32, name="mask2")
                    nc.vector.tensor_scalar(
                        mask2[:], l2[:], m2[:], None, op0=OP.is_equal
                    )
                    w1 = gpool.tile([128, 1], f32, name="w1")
                    nc.scalar.activation(
                        w1[:], m2[:], ACT.Sigmoid, bias=m1[:], scale=-1.0
                    )
                    w2 = gpool.tile([128, 1], f32, name="w2")
                    nc.vector.tensor_scalar(
                        w2[:], w1[:], -1.0, 1.0, op0=OP.mult, op1=OP.add
                    )
                    g1 = gpool.tile([128, E_SZ], f32, name="g1")
                    nc.vector.tensor_scalar(
                        g1[:], mask1[:], w1[:], None, op0=OP.mult
                    )
                    gates = gpool.tile([128, E_SZ], f32, name="gates")
                    nc.vector.tensor_scalar(
                        gates[:], mask2[:], w2[:], None, op0=OP.mult
                    )
                    nc.vector.tensor_tensor(
                        gates[:], gates[:], g1[:], op=OP.add
                    )
                    gates_t.append(gates)

                # h^T copies (DVE waits on h stop, after the gating chains)
                for tb in range(NB):
                    sl = slice(tb * 512, (tb + 1) * 512)
                    nc.vector.tensor_copy(hT_sb[:, sl], ph[tb][:])

                # ---- phase 1c-c: expand gates, gh^T = ge * h^T (bf16);
                # batched in groups of 4 tiles to cut PE<->DVE ping-pong
                for grp in range(NT // 4):
                    tts = range(grp * 4, grp * 4 + 4)
                    ptr_all = psum.tile(
                        [E_SZ, 4, 128], f32, tag="bank", name=f"ptr_g{grp}"
                    )
                    for i, tt in enumerate(tts):
                        nc.tensor.transpose(
                            ptr_all[:, i, :], gates_t[tt][:], ident_sb[:]
                        )
                    gT = gpool.tile([E_SZ, 4, 128], bf16, name="gT")
                    nc.vector.tensor_copy(gT[:], ptr_all[:])
                    pges = []
                    for i, tt in enumerate(tts):
                        pge = psum.tile(
                            [128, 128], f32, tag="bank", name=f"pge{tt}"
                        )
                        nc.tensor.matmul(
                            pge[:], sel_sb[:], gT[:, i, :], start=True, stop=True
                        )
                        pges.append(pge)
                    for i, tt in enumerate(tts):
                        ts = slice(tt * 128, (tt + 1) * 128)
                        gh32 = gpool.tile([128, 128], f32, name="gh32")
                        nc.vector.tensor_tensor(
                            gh32[:], pges[i][:], hT_sb[:, ts], op=OP.mult
                        )
                        nc.vector.tensor_copy(ghT_sb[:, tt, :], gh32[:])

                # ---- phase 2: out = x @ W^T (+ gh @ SCALING*B_cat), bf16 x1
                for ot in range(NO):
                    po = [
                        psum.tile([128, 512], f32, tag="bank", name=f"po{ot}_{i}")
                        for i in range(NT)
                    ]
                    for kcb in range(NKB):
                        w_t = wpool.tile([128, 4, 512], bf16, name="w_t")
                        eng = nc.sync if kcb % 2 == 0 else nc.scalar
                        eng.dma_start(w_t[:], wt_d[ot, kcb])
                        for g in range(4):
                            kc = kcb * 4 + g
                            wh = w_t[:, g, :]
                            for tt in range(NT):
                                xs = xh(kc)[:, tt * 128 : (tt + 1) * 128]
                                nc.tensor.matmul(
                                    po[tt][:], xs, wh, start=(kc == 0), stop=False
                                )
                    osl = slice(ot * 512, (ot + 1) * 512)
                    ob = opool.tile([128, NT, 512], bf16, name="ob")
                    for tt in range(NT):
                        nc.tensor.matmul(
                            po[tt][:],
                            ghT_sb[:, tt, :],
                            bch_sb[:, osl],
                            start=False,
                            stop=True,
                        )
                        if tt % 2 == 0:
                            nc.vector.tensor_copy(ob[:, tt, :], po[tt][:])
                        else:
                            nc.scalar.copy(ob[:, tt, :], po[tt][:])
                        # half-tile DMAs overlap the remaining evacuations
                        if tt == NT // 2 - 1:
                            nc.gpsimd.dma_start(
                                out_d[ot, :, 0 : NT // 2], ob[:, 0 : NT // 2, :]
                            )
                        elif tt == NT - 1:
                            nc.gpsimd.dma_start(
                                out_d[ot, :, NT // 2 :], ob[:, NT // 2 :, :]
                            )

            if loop_n is None:
                body()
            else:
                with tc.For_i(0, loop_n, 1) as iv:
                    body(iv)

    nc.compile()
    return nc


def _split_bf16(a):
    import ml_dtypes

    hi = a.astype(ml_dtypes.bfloat16)
    lo = (a - hi.astype(np.float32)).astype(ml_dtypes.bfloat16)
    return hi, lo


def _host_prep(x, language_ids, W, Wr, A, B):
    import ml_dtypes

    x = np.asarray(x, dtype=np.float32)
    W = np.asarray(W, dtype=np.float32)
    Wr = np.asarray(Wr, dtype=np.float32)
    A = np.asarray(A, dtype=np.float32)
    B = np.asarray(B, dtype=np.float32)
    lang = np.asarray(language_ids).astype(np.int64)

    xf = np.ascontiguousarray(x.reshape(B_SZ * T_SZ, D_SZ))

    # W^T [D,O] hi part -> [NO, NKB, 128, 4, 512] row-contiguous
    wtT = np.ascontiguousarray(W.T).reshape(NK, 128, NO, 512)  # [kc, p, ot, n]
    wh = wtT.astype(ml_dtypes.bfloat16)
    wt = np.ascontiguousarray(
        wh.reshape(NKB, 4, 128, NO, 512).transpose(3, 0, 2, 1, 4)
    )

    acat_t = np.ascontiguousarray(A.reshape(ER, D_SZ).T).reshape(NK, 128, ER)
    acat = acat_t.astype(ml_dtypes.bfloat16)

    wrT = np.ascontiguousarray(Wr.T)                       # [D, E]
    wrh, wrl = _split_bf16(wrT)
    wrcat = np.zeros((D_SZ, 40), dtype=ml_dtypes.bfloat16)
    wrcat[:, 0:E_SZ] = wrh
    wrcat[:, 32:40] = wrl
    wrcat = np.ascontiguousarray(wrcat).reshape(NK, 128, 40)

    bcat32 = (SCALING * B.transpose(0, 2, 1)).reshape(ER, O_SZ)
    bcat = bcat32.astype(ml_dtypes.bfloat16)

    sel = np.zeros((E_SZ, ER), dtype=np.float32)
    sel[np.arange(ER) // R_SZ, np.arange(ER)] = 1.0
    sel = sel.astype(ml_dtypes.bfloat16)

    in_maps = []
    for c in range(NCORES):
        shard = xf[c * TPC : (c + 1) * TPC]
        xr = np.ascontiguousarray(shard.T).reshape(NK, 128, TPC)
        xhh, xll = _split_bf16(xr)
        xt = np.ascontiguousarray(np.stack([xhh, xll], axis=0))  # [2, kc, p, t]
        b = int(lang[(c * TPC) // T_SZ])
        brow = np.zeros((E_SZ, 1), dtype=np.float32)
        if b >= 0:
            brow[b, 0] = LANG_BIAS
        in_maps.append(
            {
                "xt": xt,
                "wt": wt,
                "acat": acat,
                "wrcat": wrcat,
                "bcat": bcat,
                "biasx": brow,
                "sel": sel,
            }
        )
    return in_maps


def kernel(x, language_ids, W, Wr, A, B):
    global LAST_RESULT
    from concourse.bass_utils import run_bass_kernel_spmd

    if "nc" not in _CACHE:
        _CACHE["nc"] = _build_bass()
    nc = _CACHE["nc"]

    in_maps = _host_prep(x, language_ids, W, Wr, A, B)
    res = run_bass_kernel_spmd(nc, in_maps, core_ids=list(range(NCORES)))
    LAST_RESULT = res
    outs = [
        np.asarray(r["out"], dtype=np.float32)
        .transpose(2, 1, 0, 3)
        .reshape(TPC, O_SZ)
        for r in res.results
    ]
    return np.concatenate(outs, axis=0).reshape(B_SZ, T_SZ, O_SZ)

